# revision 1
# baseline (speedup 1.0000x reference)
"""Trainium2 Bass kernel for nn_Attention_25769804179.

Multi-head attention (B=4, S=2048, D=1024, H=16, hd=64), fp32 I/O.

Sharding: batch (4-way) x head-group (2-way, 8 heads each) over 8 NeuronCores.
Each core computes, for its batch b and head group g:
  qkv projection (its heads only), per-head softmax(q k^T / 8) v in a
  transposed-scores layout, and a partial output projection
  ctx @ W_proj[rows of g].  The host sums the two partials per batch and adds
  the bias terms.

On-chip layout (per core):
  stage 1: qT,kT [64*8, S] (head-dim on partitions; bias added per-partition)
           and v  [S, 64*8+ones] (natural layout, augmented with a ones column
           so the PV matmul also produces the softmax denominator Z)
  stage 2: per head pair: scoresT = k q^T via row-packed (K=64) fp32r matmuls,
           exp on ScalarE straight from PSUM (scale=1/8 folded in), PV + Z via
           M=65 fp32r matmuls, normalization via reciprocal + K=1 broadcast
           matmul, results written to ctxT.
  stage 3: out = ctx @ W_proj via fp32r matmuls from ctxT.

All matmuls run in float32r (single-pass fp32 mode, ~tf32 precision, 4x the
throughput of strict fp32); measured end-to-end max rel err ~4e-4.
"""

import sys
import time

sys.path.insert(0, "/opt/trn_rl_repo")

import numpy as np

B, S, D = 4, 2048, 1024
NH, HD = 16, 64
HPC = 8          # heads per core
NPAIR = HPC // 2
SCALE = HD ** -0.5
NKT = S // 128   # 16 k-tiles
NSQ = S // 512   # 4 q-tiles of 512
NDT = D // 128   # 8 d-tiles
NCORES = 8

_RUNNER = None


def _build(repeat=1):
    import concourse.mybir as mybir
    import concourse.tile as tile
    from concourse import bacc

    dt = mybir.dt
    f32, f32r, f16 = dt.float32, dt.float32r, dt.float16
    AF = mybir.ActivationFunctionType
    ALU = mybir.AluOpType

    nc = bacc.Bacc("TRN2", debug=False, enable_partition_id=False)

    xt_d = nc.dram_tensor("xt", [NSQ, 128, NDT * 512], f32r, kind="ExternalInput").ap()
    wqk_d = nc.dram_tensor("wqk", [2, NPAIR, 128, NDT * 128], f32r, kind="ExternalInput").ap()
    wv_d = nc.dram_tensor("wv", [128, NDT * 512], f32r, kind="ExternalInput").ap()
    wp_d = nc.dram_tensor("wp", [128, 4 * 1024], f32r, kind="ExternalInput").ap()
    bqk_d = nc.dram_tensor("bqk", [128, 2 * NPAIR], f32, kind="ExternalInput").ap()
    out_d = nc.dram_tensor("out", [S // 128, 128, D], f32, kind="ExternalOutput").ap()

    with tile.TileContext(nc) as tc:
        with tc.tile_pool(name="consts", bufs=1) as consts, \
             tc.tile_pool(name="wqk", bufs=2) as wqk_pool, \
             tc.tile_pool(name="xt", bufs=3) as xt_pool, \
             tc.tile_pool(name="qt", bufs=2) as qt_pool, \
             tc.tile_pool(name="kt", bufs=2) as kt_pool, \
             tc.tile_pool(name="ex", bufs=4) as ex_pool, \
             tc.tile_pool(name="rc", bufs=1) as rc_pool, \
             tc.tile_pool(name="pvs", bufs=2) as pvs_pool, \
             tc.tile_pool(name="osb", bufs=2) as out_pool, \
             tc.tile_pool(name="ps1", bufs=1, space="PSUM") as ps1, \
             tc.tile_pool(name="pssc", bufs=2, space="PSUM") as ps_sc, \
             tc.tile_pool(name="pspv", bufs=1, space="PSUM") as ps_pv:

            wv_sb = consts.tile([128, NDT * 512], f32r, name="wv_sb")
            nc.sync.dma_start(wv_sb[:], wv_d[:])
            wp_sb = consts.tile([128, 4 * 1024], f32r, name="wp_sb")
            nc.sync.dma_start(wp_sb[:], wp_d[:])
            bqk_sb = consts.tile([128, 2 * NPAIR], f32, name="bqk_sb")
            nc.sync.dma_start(bqk_sb[:], bqk_d[:])
            ones_f = consts.tile([128, 64], f32, name="ones_f")
            nc.vector.memset(ones_f[:], 1.0)
            ones_h = consts.tile([1, 64], f16, name="ones_h")
            nc.vector.tensor_copy(ones_h[:], ones_f[0:1, :])

            va = consts.tile([128, NKT * HPC * 65], f32r, name="va")
            vav = va[:].rearrange("p (k c) -> p k c", c=65)  # [128, NKT*HPC, 65]
            ctxt = consts.tile([128, NPAIR * S], f32r, name="ctxt")

            # ones column of v_aug (softmax denominator trick), written once
            nc.vector.tensor_copy(
                vav[:, :, 64:65], ones_f[:, 0:1].broadcast_to((128, NKT * HPC, 1))
            )

            def stage1_sweep(r, p):
                wq_t = wqk_pool.tile([128, NDT * 128], f32r, tag="wq", name=f"wq_{r}_{p}")
                nc.sync.dma_start(wq_t[:], wqk_d[0, p])
                wk_t = wqk_pool.tile([128, NDT * 128], f32r, tag="wk", name=f"wk_{r}_{p}")
                nc.sync.dma_start(wk_t[:], wqk_d[1, p])
                qt_t = qt_pool.tile([128, S], f32r, tag="qt", name=f"qt_{r}_{p}")
                kt_t = kt_pool.tile([128, S], f32r, tag="kt", name=f"kt_{r}_{p}")
                for c in range(NSQ):
                    xt_h = []
                    for h in range(2):
                        xh = xt_pool.tile([128, 4 * 512], f32r, tag="xt", name=f"xt_{r}_{p}_{c}_{h}")
                        nc.sync.dma_start(xh[:], xt_d[c, :, h * 2048:(h + 1) * 2048])
                        xt_h.append(xh)
                    for qk, (w_t, dst) in enumerate(((wq_t, qt_t), (wk_t, kt_t))):
                        ps = ps1.tile([128, 512], f32, tag="ps1", name=f"s1_{r}_{p}_{c}_{qk}")
                        for dtl in range(NDT):
                            nc.tensor.matmul(
                                ps[:],
                                w_t[:, dtl * 128:(dtl + 1) * 128],
                                xt_h[dtl // 4][:, (dtl % 4) * 512:(dtl % 4 + 1) * 512],
                                start=(dtl == 0), stop=(dtl == NDT - 1),
                            )
                        nc.vector.tensor_scalar_add(
                            dst[:, c * 512:(c + 1) * 512], ps[:],
                            bqk_sb[:, qk * NPAIR + p: qk * NPAIR + p + 1],
                        )
                    if p == 0:
                        for ss in range(4):
                            ktile = c * 4 + ss
                            ps = ps1.tile([128, 512], f32, tag="ps1", name=f"v_{r}_{c}_{ss}")
                            for dtl in range(NDT):
                                nc.tensor.matmul(
                                    ps[:],
                                    xt_h[dtl // 4][:, (dtl % 4) * 512 + ss * 128: (dtl % 4) * 512 + (ss + 1) * 128],
                                    wv_sb[:, dtl * 512:(dtl + 1) * 512],
                                    start=(dtl == 0), stop=(dtl == NDT - 1),
                                )
                            nc.vector.tensor_copy(
                                vav[:, ktile * HPC:(ktile + 1) * HPC, 0:64],
                                ps[:].rearrange("p (h e) -> p h e", e=64),
                            )
                return qt_t, kt_t

            def stage2_pair(r, p, qt_t, kt_t, tail=False):
                for j in range(NSQ):
                    pv_lo = ps_pv.tile([65, 512], f32, tag="pvlo", name=f"pvl_{r}_{p}_{j}")
                    pv_hi = ps_pv.tile([65, 512], f32, tag="pvhi", name=f"pvh_{r}_{p}_{j}")
                    q_sl = qt_t[:, j * 512:(j + 1) * 512]
                    for i in range(NKT):
                        sc = ps_sc.tile([128, 1024], f32, tag="sc", name=f"sc_{r}_{p}_{j}_{i}")
                        nc.tensor.matmul(sc[:, 0:512], kt_t[0:64, i * 128:(i + 1) * 128], q_sl[0:64, :])
                        nc.tensor.matmul(sc[:, 512:1024], kt_t[64:128, i * 128:(i + 1) * 128], q_sl[64:128, :])
                        ex = ex_pool.tile([128, 1024], f32r, tag="ex", name=f"ex_{r}_{p}_{j}_{i}")
                        nc.scalar.activation(ex[:], sc[:], AF.Exp, scale=SCALE)
                        nc.tensor.matmul(
                            pv_lo[:], vav[:, i * HPC + 2 * p, :], ex[:, 0:512],
                            start=(i == 0), stop=(i == NKT - 1),
                        )
                        nc.tensor.matmul(
                            pv_hi[:], vav[:, i * HPC + 2 * p + 1, :], ex[:, 512:1024],
                            start=(i == 0), stop=(i == NKT - 1),
                        )
                    # free the pv accumulators ASAP: stage to SBUF, normalize from there
                    pv_sb = pvs_pool.tile([65, 1024], f32, tag="pvs", name=f"pvs_{r}_{p}_{j}")
                    nc.vector.tensor_copy(pv_sb[:, 0:512], pv_lo[:])
                    nc.vector.tensor_copy(pv_sb[:, 512:1024], pv_hi[:])
                    rc_t = rc_pool.tile([1, 1024], f32, tag="rc", name=f"rc_{r}_{p}_{j}")
                    nc.vector.reciprocal(rc_t[0:1, 0:512], pv_sb[64:65, 0:512])
                    nc.vector.reciprocal(rc_t[0:1, 512:1024], pv_sb[64:65, 512:1024])
                    rc_h = rc_pool.tile([1, 1024], f16, tag="rch", name=f"rch_{r}_{p}_{j}")
                    nc.vector.tensor_copy(rc_h[:], rc_t[:])
                    bc_ps = ps_pv.tile([128, 512], f32, tag="bc", name=f"bc_{r}_{p}_{j}")
                    nc.tensor.matmul(bc_ps[0:64, :], ones_h[0:1, :], rc_h[0:1, 0:512])
                    nc.tensor.matmul(bc_ps[64:128, :], ones_h[0:1, :], rc_h[0:1, 512:1024])
                    cslice = ctxt[:, p * S + j * 512: p * S + (j + 1) * 512]
                    nc.vector.tensor_tensor(cslice[0:64, :], bc_ps[0:64, :], pv_sb[0:64, 0:512], ALU.mult)
                    nc.vector.tensor_tensor(cslice[64:128, :], bc_ps[64:128, :], pv_sb[0:64, 512:1024], ALU.mult)
                    if tail:
                        stage3_block(r, range(4 * j, 4 * j + 4))

            def stage3_block(r, ts_range):
                for t in ts_range:
                    o_t = out_pool.tile([128, 1024], f32, tag="o", name=f"o_{r}_{t}")
                    for ch in range(2):
                        ps = ps1.tile([128, 512], f32, tag="ps1", name=f"s3_{r}_{t}_{ch}")
                        for ft in range(NPAIR):
                            nc.tensor.matmul(
                                ps[:],
                                ctxt[:, ft * S + t * 128: ft * S + (t + 1) * 128],
                                wp_sb[:, ft * 1024 + ch * 512: ft * 1024 + (ch + 1) * 512],
                                start=(ft == 0), stop=(ft == NPAIR - 1),
                            )
                        nc.vector.tensor_copy(o_t[:, ch * 512:(ch + 1) * 512], ps[:])
                    nc.sync.dma_start(out_d[t], o_t[:])

            for r in range(repeat):
                for p in range(NPAIR):
                    qt_t, kt_t = stage1_sweep(r, p)
                    stage2_pair(r, p, qt_t, kt_t, tail=(p == NPAIR - 1))

    nc.compile()
    return nc


def _make_runner(nc):
    import jax
    import jax.core as jcore
    from jax.experimental.shard_map import shard_map
    from jax.sharding import Mesh, NamedSharding, PartitionSpec

    import concourse.mybir as mybir
    from concourse import bass2jax

    bass2jax.install_neuronx_cc_hook()

    in_names, out_names, out_avals, zero_outs = [], [], [], []
    for alloc in nc.m.functions[0].allocations:
        if not isinstance(alloc, mybir.MemoryLocationSet):
            continue
        name = alloc.memorylocations[0].name
        if alloc.kind == "ExternalInput":
            in_names.append(name)
        elif alloc.kind == "ExternalOutput":
            out_names.append(name)
            shape = tuple(alloc.tensor_shape)
            npdt = mybir.dt.np(alloc.dtype)
            out_avals.append(jcore.ShapedArray(shape, npdt))
            zero_outs.append(np.zeros(shape, npdt))
    n_params = len(in_names)
    all_names = tuple(in_names + out_names)

    def _body(*args):
        outs = bass2jax._bass_exec_p.bind(
            *args,
            out_avals=tuple(out_avals),
            in_names=all_names,
            out_names=tuple(out_names),
            lowering_input_output_aliases=(),
            sim_require_finite=True,
            sim_require_nnan=True,
            nc=nc,
        )
        return tuple(outs)

    devices = jax.devices()[:NCORES]
    mesh = Mesh(np.asarray(devices), ("core",))
    nio = n_params + len(out_names)
    sharded = jax.jit(
        shard_map(
            _body, mesh=mesh,
            in_specs=(PartitionSpec("core"),) * nio,
            out_specs=(PartitionSpec("core"),) * len(out_names),
            check_rep=False,
        ),
        keep_unused=True,
    )
    sh = NamedSharding(mesh, PartitionSpec("core"))

    def run(in_maps, reps=1):
        concat = [
            np.concatenate([np.asarray(in_maps[c][n]) for c in range(NCORES)], axis=0)
            for n in in_names
        ]
        concat += [np.concatenate([z] * NCORES, axis=0) for z in zero_outs]
        dev_in = [jax.device_put(a, sh) for a in concat]
        outs = sharded(*dev_in)
        jax.block_until_ready(outs)
        per_call = None
        if reps > 1:
            t0 = time.perf_counter()
            for _ in range(reps - 1):
                outs = sharded(*dev_in)
            jax.block_until_ready(outs)
            per_call = (time.perf_counter() - t0) / (reps - 1)
        results = []
        for c in range(NCORES):
            d = {}
            for i, n in enumerate(out_names):
                arr = np.asarray(outs[i])
                d[n] = arr.reshape((NCORES,) + out_avals[i].shape)[c]
            results.append(d)
        return results, per_call

    return run


def _get_runner():
    global _RUNNER
    if _RUNNER is None:
        from concourse import bass_utils

        nc = _build()

        def run(in_maps, reps=1):
            if reps > 1:
                return _make_runner(nc)(in_maps, reps=reps)
            res = bass_utils.run_bass_kernel_spmd(nc, in_maps, core_ids=list(range(NCORES)))
            return res.results, None

        _RUNNER = run
    return _RUNNER


def _prep_in_maps(x, W_qkv, b_qkv, W_proj):
    in_maps = []
    for core in range(NCORES):
        b, g = core // 2, core % 2
        xT = np.ascontiguousarray(x[b].T)  # [D, S]
        xt = xT.reshape(NDT, 128, NSQ, 512).transpose(2, 1, 0, 3).reshape(NSQ, 128, NDT * 512)
        wq = W_qkv[:, g * 512:(g + 1) * 512]
        wk = W_qkv[:, 1024 + g * 512:1024 + (g + 1) * 512]
        wv = W_qkv[:, 2048 + g * 512:2048 + (g + 1) * 512]
        wqk = np.stack([
            w.reshape(NDT, 128, NPAIR, 128).transpose(2, 1, 0, 3).reshape(NPAIR, 128, NDT * 128)
            for w in (wq, wk)
        ])
        wv_t = wv.reshape(NDT, 128, 512).transpose(1, 0, 2).reshape(128, NDT * 512)
        wp_t = W_proj[g * 512:(g + 1) * 512].reshape(NPAIR, 128, 1024).transpose(1, 0, 2).reshape(128, NPAIR * 1024)
        bq = b_qkv[g * 512:(g + 1) * 512].reshape(NPAIR, 128).T
        bk = b_qkv[1024 + g * 512:1024 + (g + 1) * 512].reshape(NPAIR, 128).T
        bqk = np.concatenate([bq, bk], axis=1)
        in_maps.append({
            "xt": np.ascontiguousarray(xt, np.float32),
            "wqk": np.ascontiguousarray(wqk, np.float32),
            "wv": np.ascontiguousarray(wv_t, np.float32),
            "wp": np.ascontiguousarray(wp_t, np.float32),
            "bqk": np.ascontiguousarray(bqk, np.float32),
        })
    return in_maps


def _assemble(results, b_qkv, W_proj, b_proj):
    const = (b_qkv[2048:3072].astype(np.float64) @ W_proj.astype(np.float64)).astype(np.float32) + b_proj
    parts = [results[c]["out"].reshape(S, D) for c in range(NCORES)]
    out = np.stack([parts[2 * b] + parts[2 * b + 1] + const for b in range(B)])
    return out.astype(np.float32)


def kernel(x, W_qkv, b_qkv, W_proj, b_proj, _reps=1):
    x = np.asarray(x, np.float32)
    W_qkv = np.asarray(W_qkv, np.float32)
    b_qkv = np.asarray(b_qkv, np.float32)
    W_proj = np.asarray(W_proj, np.float32)
    b_proj = np.asarray(b_proj, np.float32)
    assert x.shape == (B, S, D), x.shape
    run = _get_runner()
    in_maps = _prep_in_maps(x, W_qkv, b_qkv, W_proj)
    results, per_call = run(in_maps, reps=_reps)
    kernel.last_per_call = per_call
    return _assemble(results, b_qkv, W_proj, b_proj)


kernel.last_per_call = None



# revision 2
# speedup vs baseline: 1.0461x; 1.0461x over previous
"""Trainium2 Bass kernel for nn_Attention_25769804179 (v2).

Multi-head attention (B=4, S=2048, D=1024, H=16, hd=64), fp32 I/O.

Sharding: batch (4-way) x head-group (2-way, 8 heads each) over 8 NeuronCores.
Each core computes, for its batch b and head group g:
  qkv projection (its heads only), per-head softmax(q k^T / 8) v in a
  transposed-scores layout, and a partial output projection
  ctx @ W_proj[rows of g].  The host sums the two partials per batch and adds
  the bias terms.

v2 changes vs v1:
  - All matmul operands are fp16 (host-converted).  On TRN2 a 16-bit moving
    operand streams 2 elements/cycle and 128-col fp16 weights get FWL, so
    N=512 matmuls run ~131 ns instead of ~216 ns (fp32r).
  - x is loaded into SBUF once per iteration ([128, 16384] fp16) instead of
    being re-streamed from HBM for each head pair (saves 24 MiB of DMA).
  - Stage-1 (qkv projection) and stage-3 (out projection) matmuls are
    interleaved into stage-2's exp-bound inner loop through a filler queue,
    so the tensor engine never idles while the scalar engine computes exp.

On-chip layout (per core):
  stage 1: qT,kT [64*2, S] fp16 per pair (head-dim on partitions; bias added
           per-partition) and v_aug [S, 64*8+ones] fp16 (ones column makes the
           PV matmul also produce the softmax denominator Z)
  stage 2: per head pair: scoresT = k q^T (row-packed K=64 fp16 matmuls),
           exp on ScalarE straight from PSUM (scale=1/8 folded in, fp16 out),
           PV + Z via M=65 fp16 matmuls, normalization via reciprocal + K=1
           broadcast matmul, results written to ctxt (fp16).
  stage 3: out = ctx @ W_proj via fp16 matmuls from ctxt.
"""

import sys
import time
from collections import deque

sys.path.insert(0, "/opt/trn_rl_repo")

import numpy as np

B, S, D = 4, 2048, 1024
NH, HD = 16, 64
HPC = 8          # heads per core
NPAIR = HPC // 2
SCALE = HD ** -0.5
NKT = S // 128   # 16 k-tiles
NSQ = S // 512   # 4 q-tiles of 512
NDT = D // 128   # 8 d-tiles
NCORES = 8


def _build(repeat=1):
    import concourse.mybir as mybir
    import concourse.tile as tile
    from concourse import bacc

    dt = mybir.dt
    f32, f16 = dt.float32, dt.float16
    AF = mybir.ActivationFunctionType
    ALU = mybir.AluOpType

    nc = bacc.Bacc("TRN2", debug=False, enable_partition_id=False)

    xt_d = nc.dram_tensor("xt", [NSQ, 128, NDT * 512], f16, kind="ExternalInput").ap()
    wqk_d = nc.dram_tensor("wqk", [2, NPAIR, 128, NDT * 128], f16, kind="ExternalInput").ap()
    wv_d = nc.dram_tensor("wv", [128, NDT * 512], f16, kind="ExternalInput").ap()
    wp_d = nc.dram_tensor("wp", [128, 4 * 1024], f16, kind="ExternalInput").ap()
    bqk_d = nc.dram_tensor("bqk", [128, 2 * NPAIR], f32, kind="ExternalInput").ap()
    out_d = nc.dram_tensor("out", [S // 128, 128, D], f32, kind="ExternalOutput").ap()

    with tile.TileContext(nc) as tc:
        with tc.tile_pool(name="consts", bufs=1) as consts, \
             tc.tile_pool(name="xres", bufs=1) as x_pool, \
             tc.tile_pool(name="wqk", bufs=2) as wqk_pool, \
             tc.tile_pool(name="qt", bufs=2) as qt_pool, \
             tc.tile_pool(name="kt", bufs=2) as kt_pool, \
             tc.tile_pool(name="ex", bufs=3) as ex_pool, \
             tc.tile_pool(name="rc", bufs=2) as rc_pool, \
             tc.tile_pool(name="pvs", bufs=2) as pvs_pool, \
             tc.tile_pool(name="osb", bufs=2) as out_pool, \
             tc.tile_pool(name="ps1", bufs=2, space="PSUM") as ps1, \
             tc.tile_pool(name="pssc", bufs=2, space="PSUM") as ps_sc, \
             tc.tile_pool(name="pspv", bufs=1, space="PSUM") as ps_pv:

            wv_sb = consts.tile([128, NDT * 512], f16, name="wv_sb")
            nc.sync.dma_start(wv_sb[:], wv_d[:])
            wp_sb = consts.tile([128, 4 * 1024], f16, name="wp_sb")
            nc.sync.dma_start(wp_sb[:], wp_d[:])
            bqk_sb = consts.tile([128, 2 * NPAIR], f32, name="bqk_sb")
            nc.sync.dma_start(bqk_sb[:], bqk_d[:])
            ones_h = consts.tile([1, 64], f16, name="ones_h")
            nc.vector.memset(ones_h[:], 1.0)

            va = consts.tile([128, NKT * HPC * 65], f16, name="va")
            vav = va[:].rearrange("p (k c) -> p k c", c=65)  # [128, NKT*HPC, 65]
            ctxt = consts.tile([128, NPAIR * S], f16, name="ctxt")

            # ones column of v_aug (softmax denominator trick), written once
            nc.vector.memset(vav[:, :, 64:65], 1.0)

            for r in range(repeat):
                _emit_iter(nc, r, mybir, locals())

    nc.compile()
    return nc


def _emit_iter(nc, r, mybir, env):
    """Emit one full attention iteration with software-pipelined scheduling."""
    dt = mybir.dt
    f32, f16 = dt.float32, dt.float16
    AF = mybir.ActivationFunctionType
    ALU = mybir.AluOpType

    x_pool = env["x_pool"]; wqk_pool = env["wqk_pool"]
    qt_pool = env["qt_pool"]; kt_pool = env["kt_pool"]
    ex_pool = env["ex_pool"]; rc_pool = env["rc_pool"]
    pvs_pool = env["pvs_pool"]; out_pool = env["out_pool"]
    ps1 = env["ps1"]; ps_sc = env["ps_sc"]; ps_pv = env["ps_pv"]
    wv_sb = env["wv_sb"]; wp_sb = env["wp_sb"]; bqk_sb = env["bqk_sb"]
    ones_h = env["ones_h"]; vav = env["vav"]; ctxt = env["ctxt"]
    xt_d = env["xt_d"]; wqk_d = env["wqk_d"]; out_d = env["out_d"]

    # x resident in SBUF: [128, c*4096 + d*512 + s] fp16
    x_sb = x_pool.tile([128, NSQ * NDT * 512], f16, tag="x", name=f"x_{r}")
    for c in range(NSQ):
        nc.sync.dma_start(x_sb[:, c * 4096:(c + 1) * 4096], xt_d[c])

    def x_slice(c, d):
        return x_sb[:, c * 4096 + d * 512: c * 4096 + (d + 1) * 512]

    # ---- stage 1 generators (yield thunks; each ~1-2 PE instructions) ----

    def gen_qk(p):
        """QKV projection for pair p's q and k heads."""
        wq_t = wqk_pool.tile([128, NDT * 128], f16, tag="wq", name=f"wq_{r}_{p}")
        nc.sync.dma_start(wq_t[:], wqk_d[0, p])
        wk_t = wqk_pool.tile([128, NDT * 128], f16, tag="wk", name=f"wk_{r}_{p}")
        nc.sync.dma_start(wk_t[:], wqk_d[1, p])
        qt_t = qt_pool.tile([128, S], f16, tag="qt", name=f"qt_{r}_{p}")
        kt_t = kt_pool.tile([128, S], f16, tag="kt", name=f"kt_{r}_{p}")
        for c in range(NSQ):
            for qk, (w_t, dst) in enumerate(((wq_t, qt_t), (wk_t, kt_t))):
                ps = ps1.tile([128, 512], f32, tag="ps1", name=f"s1_{r}_{p}_{c}_{qk}")
                for d0 in range(0, NDT, 2):
                    def mm(d0=d0, ps=ps, w_t=w_t, c=c):
                        for dtl in (d0, d0 + 1):
                            nc.tensor.matmul(
                                ps[:], w_t[:, dtl * 128:(dtl + 1) * 128], x_slice(c, dtl),
                                start=(dtl == 0), stop=(dtl == NDT - 1),
                            )
                    yield mm
                def cp(ps=ps, dst=dst, c=c, qk=qk, p=p):
                    nc.vector.tensor_scalar_add(
                        dst[:, c * 512:(c + 1) * 512], ps[:],
                        bqk_sb[:, qk * NPAIR + p: qk * NPAIR + p + 1],
                    )
                yield cp
        return qt_t, kt_t

    def gen_v(c):
        """V projection for query block c (k-tiles 4c..4c+3), all 8 heads."""
        for ss in range(4):
            ktile = c * 4 + ss
            ps = ps1.tile([128, 512], f32, tag="ps1", name=f"v_{r}_{c}_{ss}")
            for d0 in range(0, NDT, 2):
                def mm(d0=d0, ps=ps, c=c, ss=ss):
                    for dtl in (d0, d0 + 1):
                        nc.tensor.matmul(
                            ps[:],
                            x_sb[:, c * 4096 + dtl * 512 + ss * 128:
                                 c * 4096 + dtl * 512 + (ss + 1) * 128],
                            wv_sb[:, dtl * 512:(dtl + 1) * 512],
                            start=(dtl == 0), stop=(dtl == NDT - 1),
                        )
                yield mm
            def cp(ps=ps, ktile=ktile):
                nc.vector.tensor_copy(
                    vav[:, ktile * HPC:(ktile + 1) * HPC, 0:64],
                    ps[:].rearrange("p (h e) -> p h e", e=64),
                )
            yield cp

    def gen_stage3(t):
        """Out projection for q-tile t (128 queries)."""
        o_t = out_pool.tile([128, 1024], f32, tag="o", name=f"o_{r}_{t}")
        for ch in range(2):
            ps = ps1.tile([128, 512], f32, tag="ps1", name=f"s3_{r}_{t}_{ch}")
            for f0 in range(0, NPAIR, 2):
                def mm(f0=f0, ps=ps, ch=ch, t=t):
                    for ft in (f0, f0 + 1):
                        nc.tensor.matmul(
                            ps[:],
                            ctxt[:, ft * S + t * 128: ft * S + (t + 1) * 128],
                            wp_sb[:, ft * 1024 + ch * 512: ft * 1024 + (ch + 1) * 512],
                            start=(ft == 0), stop=(ft == NPAIR - 1),
                        )
                yield mm
            def cp(ps=ps, o_t=o_t, ch=ch):
                nc.vector.tensor_copy(o_t[:, ch * 512:(ch + 1) * 512], ps[:])
            yield cp
        def dma(o_t=o_t, t=t):
            nc.sync.dma_start(out_d[t], o_t[:])
        yield dma

    # ---- filler queue ----
    fillers = deque()

    def push(gen):
        fillers.extend(gen)

    def fill(n):
        for _ in range(n):
            if not fillers:
                return
            fillers.popleft()()

    def drain():
        while fillers:
            fillers.popleft()()

    # ---- prelude: q/k for pair 0, v for block 0, inline ----
    qk0 = gen_qk(0)
    qt_t, kt_t = None, None
    try:
        while True:
            next(qk0)()
    except StopIteration as e:
        qt_t, kt_t = e.value
    for th in gen_v(0):
        th()
    # v blocks 1-3 become the first fillers (paced ahead of the pv consumers)
    for c in range(1, NSQ):
        push(gen_v(c))

    # ---- main loop over head pairs ----
    next_qt = {}
    for p in range(NPAIR):
        if p < NPAIR - 1:
            g = gen_qk(p + 1)
            res = []

            def qk_wrap(g=g, res=res):
                try:
                    while True:
                        yield next(g)
                except StopIteration as e:
                    res.append(e.value)
            push(qk_wrap())
            next_qt[p + 1] = res

        for j in range(NSQ):
            pv_lo = ps_pv.tile([65, 512], f32, tag="pvlo", name=f"pvl_{r}_{p}_{j}")
            pv_hi = ps_pv.tile([65, 512], f32, tag="pvhi", name=f"pvh_{r}_{p}_{j}")
            q_sl = qt_t[:, j * 512:(j + 1) * 512]
            for i in range(NKT):
                sc = ps_sc.tile([128, 1024], f32, tag="sc", name=f"sc_{r}_{p}_{j}_{i}")
                nc.tensor.matmul(sc[:, 0:512], kt_t[0:64, i * 128:(i + 1) * 128], q_sl[0:64, :])
                nc.tensor.matmul(sc[:, 512:1024], kt_t[64:128, i * 128:(i + 1) * 128], q_sl[64:128, :])
                ex = ex_pool.tile([128, 1024], f16, tag="ex", name=f"ex_{r}_{p}_{j}_{i}")
                nc.scalar.activation(ex[:], sc[:], AF.Exp, scale=SCALE)
                nc.tensor.matmul(
                    pv_lo[:], vav[:, i * HPC + 2 * p, :], ex[:, 0:512],
                    start=(i == 0), stop=(i == NKT - 1),
                )
                nc.tensor.matmul(
                    pv_hi[:], vav[:, i * HPC + 2 * p + 1, :], ex[:, 512:1024],
                    start=(i == 0), stop=(i == NKT - 1),
                )
                # pace fillers: v-blocks must stay ahead of the pv reads
                fill(10 if (p == 0 and j == 0) else 2)
            # normalize: stage pv to SBUF, 1/Z, broadcast via K=1 matmul
            pv_sb = pvs_pool.tile([65, 1024], f32, tag="pvs", name=f"pvs_{r}_{p}_{j}")
            nc.vector.tensor_copy(pv_sb[:, 0:512], pv_lo[:])
            nc.vector.tensor_copy(pv_sb[:, 512:1024], pv_hi[:])
            rc_t = rc_pool.tile([1, 1024], f32, tag="rc", name=f"rc_{r}_{p}_{j}")
            nc.vector.reciprocal(rc_t[0:1, 0:512], pv_sb[64:65, 0:512])
            nc.vector.reciprocal(rc_t[0:1, 512:1024], pv_sb[64:65, 512:1024])
            rc_h = rc_pool.tile([1, 1024], f16, tag="rch", name=f"rch_{r}_{p}_{j}")
            nc.vector.tensor_copy(rc_h[:], rc_t[:])
            bc_ps = ps_pv.tile([128, 512], f32, tag="pvlo", name=f"bc_{r}_{p}_{j}")
            nc.tensor.matmul(bc_ps[0:64, :], ones_h[0:1, :], rc_h[0:1, 0:512])
            nc.tensor.matmul(bc_ps[64:128, :], ones_h[0:1, :], rc_h[0:1, 512:1024])
            cslice = ctxt[:, p * S + j * 512: p * S + (j + 1) * 512]
            nc.vector.tensor_tensor(cslice[0:64, :], bc_ps[0:64, :], pv_sb[0:64, 0:512], ALU.mult)
            nc.vector.tensor_tensor(cslice[64:128, :], bc_ps[64:128, :], pv_sb[0:64, 512:1024], ALU.mult)
            if p == NPAIR - 1:
                # ctxt rows for q-tiles 4j..4j+3 now complete: out projection
                for t in range(4 * j, 4 * j + 4):
                    push(gen_stage3(t))

        if p + 1 in next_qt:
            drain()  # ensure pair p+1's q/k are fully emitted
            qt_t, kt_t = next_qt[p + 1][0]

    drain()


def _make_runner(nc):
    import jax
    import jax.core as jcore
    from jax.experimental.shard_map import shard_map
    from jax.sharding import Mesh, NamedSharding, PartitionSpec

    import concourse.mybir as mybir
    from concourse import bass2jax

    bass2jax.install_neuronx_cc_hook()

    in_names, out_names, out_avals, zero_outs = [], [], [], []
    for alloc in nc.m.functions[0].allocations:
        if not isinstance(alloc, mybir.MemoryLocationSet):
            continue
        name = alloc.memorylocations[0].name
        if alloc.kind == "ExternalInput":
            in_names.append(name)
        elif alloc.kind == "ExternalOutput":
            out_names.append(name)
            shape = tuple(alloc.tensor_shape)
            npdt = mybir.dt.np(alloc.dtype)
            out_avals.append(jcore.ShapedArray(shape, npdt))
            zero_outs.append(np.zeros(shape, npdt))
    n_params = len(in_names)
    all_names = tuple(in_names + out_names)

    def _body(*args):
        outs = bass2jax._bass_exec_p.bind(
            *args,
            out_avals=tuple(out_avals),
            in_names=all_names,
            out_names=tuple(out_names),
            lowering_input_output_aliases=(),
            sim_require_finite=True,
            sim_require_nnan=True,
            nc=nc,
        )
        return tuple(outs)

    devices = jax.devices()[:NCORES]
    mesh = Mesh(np.asarray(devices), ("core",))
    nio = n_params + len(out_names)
    sharded = jax.jit(
        shard_map(
            _body, mesh=mesh,
            in_specs=(PartitionSpec("core"),) * nio,
            out_specs=(PartitionSpec("core"),) * len(out_names),
            check_rep=False,
        ),
        keep_unused=True,
    )
    sh = NamedSharding(mesh, PartitionSpec("core"))

    def run(in_maps, reps=1):
        concat = [
            np.concatenate([np.asarray(in_maps[c][n]) for c in range(NCORES)], axis=0)
            for n in in_names
        ]
        concat += [np.concatenate([z] * NCORES, axis=0) for z in zero_outs]
        dev_in = [jax.device_put(a, sh) for a in concat]
        outs = sharded(*dev_in)
        jax.block_until_ready(outs)
        per_call = None
        if reps > 1:
            t0 = time.perf_counter()
            for _ in range(reps - 1):
                outs = sharded(*dev_in)
            jax.block_until_ready(outs)
            per_call = (time.perf_counter() - t0) / (reps - 1)
        results = []
        for c in range(NCORES):
            d = {}
            for i, n in enumerate(out_names):
                arr = np.asarray(outs[i])
                d[n] = arr.reshape((NCORES,) + out_avals[i].shape)[c]
            results.append(d)
        return results, per_call

    return run


_RUNNER = None


def _get_runner():
    global _RUNNER
    if _RUNNER is None:
        from concourse import bass_utils

        nc = _build()

        def run(in_maps, reps=1):
            if reps > 1:
                return _make_runner(nc)(in_maps, reps=reps)
            res = bass_utils.run_bass_kernel_spmd(nc, in_maps, core_ids=list(range(NCORES)))
            return res.results, None

        _RUNNER = run
    return _RUNNER


def _prep_in_maps(x, W_qkv, b_qkv, W_proj):
    in_maps = []
    for core in range(NCORES):
        b, g = core // 2, core % 2
        xT = np.ascontiguousarray(x[b].T)  # [D, S]
        xt = xT.reshape(NDT, 128, NSQ, 512).transpose(2, 1, 0, 3).reshape(NSQ, 128, NDT * 512)
        wq = W_qkv[:, g * 512:(g + 1) * 512]
        wk = W_qkv[:, 1024 + g * 512:1024 + (g + 1) * 512]
        wv = W_qkv[:, 2048 + g * 512:2048 + (g + 1) * 512]
        wqk = np.stack([
            w.reshape(NDT, 128, NPAIR, 128).transpose(2, 1, 0, 3).reshape(NPAIR, 128, NDT * 128)
            for w in (wq, wk)
        ])
        wv_t = wv.reshape(NDT, 128, 512).transpose(1, 0, 2).reshape(128, NDT * 512)
        wp_t = W_proj[g * 512:(g + 1) * 512].reshape(NPAIR, 128, 1024).transpose(1, 0, 2).reshape(128, NPAIR * 1024)
        bq = b_qkv[g * 512:(g + 1) * 512].reshape(NPAIR, 128).T
        bk = b_qkv[1024 + g * 512:1024 + (g + 1) * 512].reshape(NPAIR, 128).T
        bqk = np.concatenate([bq, bk], axis=1)
        in_maps.append({
            "xt": np.ascontiguousarray(xt, np.float16),
            "wqk": np.ascontiguousarray(wqk, np.float16),
            "wv": np.ascontiguousarray(wv_t, np.float16),
            "wp": np.ascontiguousarray(wp_t, np.float16),
            "bqk": np.ascontiguousarray(bqk, np.float32),
        })
    return in_maps


def _assemble(results, b_qkv, W_proj, b_proj):
    const = (b_qkv[2048:3072].astype(np.float64) @ W_proj.astype(np.float64)).astype(np.float32) + b_proj
    parts = [results[c]["out"].reshape(S, D) for c in range(NCORES)]
    out = np.stack([parts[2 * b] + parts[2 * b + 1] + const for b in range(B)])
    return out.astype(np.float32)


def kernel(x, W_qkv, b_qkv, W_proj, b_proj, _reps=1):
    x = np.asarray(x, np.float32)
    W_qkv = np.asarray(W_qkv, np.float32)
    b_qkv = np.asarray(b_qkv, np.float32)
    W_proj = np.asarray(W_proj, np.float32)
    b_proj = np.asarray(b_proj, np.float32)
    assert x.shape == (B, S, D), x.shape
    run = _get_runner()
    in_maps = _prep_in_maps(x, W_qkv, b_qkv, W_proj)
    results, per_call = run(in_maps, reps=_reps)
    kernel.last_per_call = per_call
    return _assemble(results, b_qkv, W_proj, b_proj)


kernel.last_per_call = None


# revision 4
# speedup vs baseline: 1.3430x; 1.2838x over previous
"""Trainium2 Bass kernel for nn_Attention_25769804179.

Multi-head attention (B=4, S=2048, D=1024, H=16, hd=64), fp32 I/O.

Sharding: batch (4-way) x head-group (2-way, 8 heads each) over 8 NeuronCores.
Each core computes, for its batch b and head group g:
  qkv projection (its heads only), per-head softmax(q k^T / 8) v in a
  transposed-scores layout, and a partial output projection
  ctx @ W_proj[rows of g].  The host sums the two partials per batch and adds
  the bias terms.

Key optimizations vs the f32r baseline:
  - All matmul operands are fp16 (host-converted); fp16 chains measure
    ~215-245 ns/MM at N=512 vs ~260 for f32r, and the row-packed K=64 score
    pairs (disjoint PSUM banks) run concurrently at ~111 ns/MM.
  - x is loaded into SBUF once per iteration ([128, 16384] fp16) instead of
    being re-streamed from HBM for each head pair (saves 24 MiB of DMA).
  - Stage-1 (qkv projection) and stage-3 (out projection) matmuls are
    interleaved into stage-2's exp-bound inner loop through a filler queue,
    so the tensor engine never idles while the scalar engine computes exp.
  - scores(i+1) is emitted before pv(i) so the in-order PE stream never
    stalls behind a pv matmul waiting on exp(i); 1/Z reciprocals read PSUM
    directly so the broadcast matmul is not gated on the DVE staging chain.

On-chip layout (per core):
  stage 1: qT,kT [64*2, S] fp16 per pair (head-dim on partitions; bias added
           per-partition) and v_aug [S, 64*8+ones] fp16 (ones column makes the
           PV matmul also produce the softmax denominator Z)
  stage 2: per head pair: scoresT = k q^T (row-packed K=64 fp16 matmuls),
           exp on ScalarE straight from PSUM (scale=1/8 folded in, fp16 out),
           PV + Z via M=65 fp16 matmuls, normalization via reciprocal + K=1
           broadcast matmul, results written to ctxt (fp16).
  stage 3: out = ctx @ W_proj via fp16 matmuls from ctxt.
"""

import sys
import time
from collections import deque

sys.path.insert(0, "/opt/trn_rl_repo")

import numpy as np

B, S, D = 4, 2048, 1024
NH, HD = 16, 64
HPC = 8          # heads per core
NPAIR = HPC // 2
SCALE = HD ** -0.5
NKT = S // 128   # 16 k-tiles
NSQ = S // 512   # 4 q-tiles of 512
NDT = D // 128   # 8 d-tiles
NCORES = 8


def _build(repeat=1):
    import concourse.mybir as mybir
    import concourse.tile as tile
    from concourse import bacc

    dt = mybir.dt
    f32, f16 = dt.float32, dt.float16
    AF = mybir.ActivationFunctionType
    ALU = mybir.AluOpType

    nc = bacc.Bacc("TRN2", debug=False, enable_partition_id=False)

    xt_d = nc.dram_tensor("xt", [NSQ, 128, NDT * 512], f16, kind="ExternalInput").ap()
    wqk_d = nc.dram_tensor("wqk", [2, NPAIR, 128, NDT * 128], f16, kind="ExternalInput").ap()
    wv_d = nc.dram_tensor("wv", [128, NDT * 512], f16, kind="ExternalInput").ap()
    wp_d = nc.dram_tensor("wp", [128, 4 * 1024], f16, kind="ExternalInput").ap()
    bqk_d = nc.dram_tensor("bqk", [128, 2 * NPAIR], f32, kind="ExternalInput").ap()
    out_d = nc.dram_tensor("out", [S // 128, 128, D], f32, kind="ExternalOutput").ap()

    with tile.TileContext(nc) as tc:
        with tc.tile_pool(name="consts", bufs=1) as consts, \
             tc.tile_pool(name="xres", bufs=1) as x_pool, \
             tc.tile_pool(name="wqk", bufs=2) as wqk_pool, \
             tc.tile_pool(name="qt", bufs=2) as qt_pool, \
             tc.tile_pool(name="kt", bufs=2) as kt_pool, \
             tc.tile_pool(name="ex", bufs=3) as ex_pool, \
             tc.tile_pool(name="rc", bufs=2) as rc_pool, \
             tc.tile_pool(name="pvs", bufs=2) as pvs_pool, \
             tc.tile_pool(name="osb", bufs=2) as out_pool, \
             tc.tile_pool(name="ps1", bufs=2, space="PSUM") as ps1, \
             tc.tile_pool(name="pssc", bufs=2, space="PSUM") as ps_sc, \
             tc.tile_pool(name="pspv", bufs=1, space="PSUM") as ps_pv:

            wv_sb = consts.tile([128, NDT * 512], f16, name="wv_sb")
            nc.sync.dma_start(wv_sb[:], wv_d[:])
            wp_sb = consts.tile([128, 4 * 1024], f16, name="wp_sb")
            nc.sync.dma_start(wp_sb[:], wp_d[:])
            bqk_sb = consts.tile([128, 2 * NPAIR], f32, name="bqk_sb")
            nc.sync.dma_start(bqk_sb[:], bqk_d[:])
            ones_h = consts.tile([1, 64], f16, name="ones_h")
            nc.vector.memset(ones_h[:], 1.0)

            va = consts.tile([128, NKT * HPC * 65], f16, name="va")
            vav = va[:].rearrange("p (k c) -> p k c", c=65)  # [128, NKT*HPC, 65]
            ctxt = consts.tile([128, NPAIR * S], f16, name="ctxt")

            # ones column of v_aug (softmax denominator trick), written once
            nc.vector.memset(vav[:, :, 64:65], 1.0)

            for r in range(repeat):
                _emit_iter(nc, r, mybir, locals())

    nc.compile()
    return nc


def _emit_iter(nc, r, mybir, env):
    """Emit one full attention iteration with software-pipelined scheduling."""
    dt = mybir.dt
    f32, f16 = dt.float32, dt.float16
    AF = mybir.ActivationFunctionType
    ALU = mybir.AluOpType

    x_pool = env["x_pool"]; wqk_pool = env["wqk_pool"]
    qt_pool = env["qt_pool"]; kt_pool = env["kt_pool"]
    ex_pool = env["ex_pool"]; rc_pool = env["rc_pool"]
    pvs_pool = env["pvs_pool"]; out_pool = env["out_pool"]
    ps1 = env["ps1"]; ps_sc = env["ps_sc"]; ps_pv = env["ps_pv"]
    wv_sb = env["wv_sb"]; wp_sb = env["wp_sb"]; bqk_sb = env["bqk_sb"]
    ones_h = env["ones_h"]; vav = env["vav"]; ctxt = env["ctxt"]
    xt_d = env["xt_d"]; wqk_d = env["wqk_d"]; out_d = env["out_d"]

    # x resident in SBUF: [128, c*4096 + d*512 + s] fp16
    x_sb = x_pool.tile([128, NSQ * NDT * 512], f16, tag="x", name=f"x_{r}")
    for c in range(NSQ):
        nc.sync.dma_start(x_sb[:, c * 4096:(c + 1) * 4096], xt_d[c])

    def x_slice(c, d):
        return x_sb[:, c * 4096 + d * 512: c * 4096 + (d + 1) * 512]

    # ---- stage 1 generators (yield thunks; each ~1-2 PE instructions) ----

    def gen_qk(p):
        """QKV projection for pair p's q and k heads."""
        wq_t = wqk_pool.tile([128, NDT * 128], f16, tag="wq", name=f"wq_{r}_{p}")
        nc.sync.dma_start(wq_t[:], wqk_d[0, p])
        wk_t = wqk_pool.tile([128, NDT * 128], f16, tag="wk", name=f"wk_{r}_{p}")
        nc.sync.dma_start(wk_t[:], wqk_d[1, p])
        qt_t = qt_pool.tile([128, S], f16, tag="qt", name=f"qt_{r}_{p}")
        kt_t = kt_pool.tile([128, S], f16, tag="kt", name=f"kt_{r}_{p}")
        for c in range(NSQ):
            for qk, (w_t, dst) in enumerate(((wq_t, qt_t), (wk_t, kt_t))):
                ps = ps1.tile([128, 512], f32, tag="ps1", name=f"s1_{r}_{p}_{c}_{qk}")
                for d0 in range(0, NDT, 2):
                    def mm(d0=d0, ps=ps, w_t=w_t, c=c):
                        for dtl in (d0, d0 + 1):
                            nc.tensor.matmul(
                                ps[:], w_t[:, dtl * 128:(dtl + 1) * 128], x_slice(c, dtl),
                                start=(dtl == 0), stop=(dtl == NDT - 1),
                            )
                    yield mm
                def cp(ps=ps, dst=dst, c=c, qk=qk, p=p):
                    nc.vector.tensor_scalar_add(
                        dst[:, c * 512:(c + 1) * 512], ps[:],
                        bqk_sb[:, qk * NPAIR + p: qk * NPAIR + p + 1],
                    )
                yield cp
        return qt_t, kt_t

    def gen_v(c):
        """V projection for query block c (k-tiles 4c..4c+3), all 8 heads."""
        for ss in range(4):
            ktile = c * 4 + ss
            ps = ps1.tile([128, 512], f32, tag="ps1", name=f"v_{r}_{c}_{ss}")
            for d0 in range(0, NDT, 2):
                def mm(d0=d0, ps=ps, c=c, ss=ss):
                    for dtl in (d0, d0 + 1):
                        nc.tensor.matmul(
                            ps[:],
                            x_sb[:, c * 4096 + dtl * 512 + ss * 128:
                                 c * 4096 + dtl * 512 + (ss + 1) * 128],
                            wv_sb[:, dtl * 512:(dtl + 1) * 512],
                            start=(dtl == 0), stop=(dtl == NDT - 1),
                        )
                yield mm
            def cp(ps=ps, ktile=ktile):
                nc.vector.tensor_copy(
                    vav[:, ktile * HPC:(ktile + 1) * HPC, 0:64],
                    ps[:].rearrange("p (h e) -> p h e", e=64),
                )
            yield cp

    def gen_stage3(t):
        """Out projection for q-tile t (128 queries)."""
        o_t = out_pool.tile([128, 1024], f32, tag="o", name=f"o_{r}_{t}")
        for ch in range(2):
            ps = ps1.tile([128, 512], f32, tag="ps1", name=f"s3_{r}_{t}_{ch}")
            for f0 in range(0, NPAIR, 2):
                def mm(f0=f0, ps=ps, ch=ch, t=t):
                    for ft in (f0, f0 + 1):
                        nc.tensor.matmul(
                            ps[:],
                            ctxt[:, ft * S + t * 128: ft * S + (t + 1) * 128],
                            wp_sb[:, ft * 1024 + ch * 512: ft * 1024 + (ch + 1) * 512],
                            start=(ft == 0), stop=(ft == NPAIR - 1),
                        )
                yield mm
            def cp(ps=ps, o_t=o_t, ch=ch):
                nc.vector.tensor_copy(o_t[:, ch * 512:(ch + 1) * 512], ps[:])
            yield cp
        def dma(o_t=o_t, t=t):
            nc.sync.dma_start(out_d[t], o_t[:])
        yield dma

    # ---- filler queue ----
    fillers = deque()

    def push(gen):
        fillers.extend(gen)

    def fill(n):
        for _ in range(n):
            if not fillers:
                return
            fillers.popleft()()

    def drain():
        while fillers:
            fillers.popleft()()

    # ---- prelude: q/k for pair 0, v for block 0, inline ----
    qk0 = gen_qk(0)
    qt_t, kt_t = None, None
    try:
        while True:
            next(qk0)()
    except StopIteration as e:
        qt_t, kt_t = e.value
    for th in gen_v(0):
        th()
    # v blocks 1-3 become the first fillers (paced ahead of the pv consumers)
    for c in range(1, NSQ):
        push(gen_v(c))

    # ---- main loop over head pairs ----
    next_qt = {}
    for p in range(NPAIR):
        if p < NPAIR - 1:
            g = gen_qk(p + 1)
            res = []

            def qk_wrap(g=g, res=res):
                try:
                    while True:
                        yield next(g)
                except StopIteration as e:
                    res.append(e.value)
            push(qk_wrap())
            next_qt[p + 1] = res

        for j in range(NSQ):
            pv_lo = ps_pv.tile([65, 512], f32, tag="pvlo", name=f"pvl_{r}_{p}_{j}")
            pv_hi = ps_pv.tile([65, 512], f32, tag="pvhi", name=f"pvh_{r}_{p}_{j}")
            q_sl = qt_t[:, j * 512:(j + 1) * 512]

            def emit_sc(i):
                sc = ps_sc.tile([128, 1024], f32, tag="sc", name=f"sc_{r}_{p}_{j}_{i}")
                nc.tensor.matmul(sc[:, 0:512], kt_t[0:64, i * 128:(i + 1) * 128], q_sl[0:64, :])
                nc.tensor.matmul(sc[:, 512:1024], kt_t[64:128, i * 128:(i + 1) * 128], q_sl[64:128, :])
                return sc

            # scores are emitted one iteration ahead of pv so the in-order PE
            # stream never sits behind a pv that waits on exp(i)
            sc_next = emit_sc(0)
            for i in range(NKT):
                sc = sc_next
                ex = ex_pool.tile([128, 1024], f16, tag="ex", name=f"ex_{r}_{p}_{j}_{i}")
                nc.scalar.activation(ex[:], sc[:], AF.Exp, scale=SCALE)
                if i + 1 < NKT:
                    sc_next = emit_sc(i + 1)
                nc.tensor.matmul(
                    pv_lo[:], vav[:, i * HPC + 2 * p, :], ex[:, 0:512],
                    start=(i == 0), stop=(i == NKT - 1),
                )
                nc.tensor.matmul(
                    pv_hi[:], vav[:, i * HPC + 2 * p + 1, :], ex[:, 512:1024],
                    start=(i == 0), stop=(i == NKT - 1),
                )
                # pace fillers: v-blocks must stay ahead of the pv reads
                fill(10 if (p == 0 and j == 0) else 2)
            # normalize: 1/Z straight from PSUM (so rc_h is ready early for the
            # bc matmul), then stage pv to SBUF
            rc_t = rc_pool.tile([1, 1024], f32, tag="rc", name=f"rc_{r}_{p}_{j}")
            nc.vector.reciprocal(rc_t[0:1, 0:512], pv_lo[64:65, :])
            nc.vector.reciprocal(rc_t[0:1, 512:1024], pv_hi[64:65, :])
            rc_h = rc_pool.tile([1, 1024], f16, tag="rch", name=f"rch_{r}_{p}_{j}")
            nc.vector.tensor_copy(rc_h[:], rc_t[:])
            pv_sb = pvs_pool.tile([65, 1024], f32, tag="pvs", name=f"pvs_{r}_{p}_{j}")
            nc.vector.tensor_copy(pv_sb[:, 0:512], pv_lo[:])
            nc.vector.tensor_copy(pv_sb[:, 512:1024], pv_hi[:])
            bc_ps = ps_pv.tile([128, 512], f32, tag="pvlo", name=f"bc_{r}_{p}_{j}")
            nc.tensor.matmul(bc_ps[0:64, :], ones_h[0:1, :], rc_h[0:1, 0:512])
            nc.tensor.matmul(bc_ps[64:128, :], ones_h[0:1, :], rc_h[0:1, 512:1024])
            cslice = ctxt[:, p * S + j * 512: p * S + (j + 1) * 512]
            nc.vector.tensor_tensor(cslice[0:64, :], bc_ps[0:64, :], pv_sb[0:64, 0:512], ALU.mult)
            nc.vector.tensor_tensor(cslice[64:128, :], bc_ps[64:128, :], pv_sb[0:64, 512:1024], ALU.mult)
            if p == NPAIR - 1:
                # ctxt rows for q-tiles 4j..4j+3 now complete: out projection
                for t in range(4 * j, 4 * j + 4):
                    push(gen_stage3(t))

        if p + 1 in next_qt:
            drain()  # ensure pair p+1's q/k are fully emitted
            qt_t, kt_t = next_qt[p + 1][0]

    drain()


def _make_runner(nc):
    import jax
    import jax.core as jcore
    from jax.experimental.shard_map import shard_map
    from jax.sharding import Mesh, NamedSharding, PartitionSpec

    import concourse.mybir as mybir
    from concourse import bass2jax

    bass2jax.install_neuronx_cc_hook()

    in_names, out_names, out_avals, zero_outs = [], [], [], []
    for alloc in nc.m.functions[0].allocations:
        if not isinstance(alloc, mybir.MemoryLocationSet):
            continue
        name = alloc.memorylocations[0].name
        if alloc.kind == "ExternalInput":
            in_names.append(name)
        elif alloc.kind == "ExternalOutput":
            out_names.append(name)
            shape = tuple(alloc.tensor_shape)
            npdt = mybir.dt.np(alloc.dtype)
            out_avals.append(jcore.ShapedArray(shape, npdt))
            zero_outs.append(np.zeros(shape, npdt))
    n_params = len(in_names)
    all_names = tuple(in_names + out_names)

    def _body(*args):
        outs = bass2jax._bass_exec_p.bind(
            *args,
            out_avals=tuple(out_avals),
            in_names=all_names,
            out_names=tuple(out_names),
            lowering_input_output_aliases=(),
            sim_require_finite=True,
            sim_require_nnan=True,
            nc=nc,
        )
        return tuple(outs)

    devices = jax.devices()[:NCORES]
    mesh = Mesh(np.asarray(devices), ("core",))
    nio = n_params + len(out_names)
    sharded = jax.jit(
        shard_map(
            _body, mesh=mesh,
            in_specs=(PartitionSpec("core"),) * nio,
            out_specs=(PartitionSpec("core"),) * len(out_names),
            check_rep=False,
        ),
        keep_unused=True,
    )
    sh = NamedSharding(mesh, PartitionSpec("core"))

    def run(in_maps, reps=1):
        concat = [
            np.concatenate([np.asarray(in_maps[c][n]) for c in range(NCORES)], axis=0)
            for n in in_names
        ]
        concat += [np.concatenate([z] * NCORES, axis=0) for z in zero_outs]
        dev_in = [jax.device_put(a, sh) for a in concat]
        outs = sharded(*dev_in)
        jax.block_until_ready(outs)
        per_call = None
        if reps > 1:
            t0 = time.perf_counter()
            for _ in range(reps - 1):
                outs = sharded(*dev_in)
            jax.block_until_ready(outs)
            per_call = (time.perf_counter() - t0) / (reps - 1)
        results = []
        for c in range(NCORES):
            d = {}
            for i, n in enumerate(out_names):
                arr = np.asarray(outs[i])
                d[n] = arr.reshape((NCORES,) + out_avals[i].shape)[c]
            results.append(d)
        return results, per_call

    return run


_RUNNER = None


def _get_runner():
    global _RUNNER
    if _RUNNER is None:
        from concourse import bass_utils

        nc = _build()

        def run(in_maps, reps=1):
            if reps > 1:
                return _make_runner(nc)(in_maps, reps=reps)
            res = bass_utils.run_bass_kernel_spmd(nc, in_maps, core_ids=list(range(NCORES)))
            return res.results, None

        _RUNNER = run
    return _RUNNER


def _prep_in_maps(x, W_qkv, b_qkv, W_proj):
    in_maps = []
    for core in range(NCORES):
        b, g = core // 2, core % 2
        xT = np.ascontiguousarray(x[b].T)  # [D, S]
        xt = xT.reshape(NDT, 128, NSQ, 512).transpose(2, 1, 0, 3).reshape(NSQ, 128, NDT * 512)
        wq = W_qkv[:, g * 512:(g + 1) * 512]
        wk = W_qkv[:, 1024 + g * 512:1024 + (g + 1) * 512]
        wv = W_qkv[:, 2048 + g * 512:2048 + (g + 1) * 512]
        wqk = np.stack([
            w.reshape(NDT, 128, NPAIR, 128).transpose(2, 1, 0, 3).reshape(NPAIR, 128, NDT * 128)
            for w in (wq, wk)
        ])
        wv_t = wv.reshape(NDT, 128, 512).transpose(1, 0, 2).reshape(128, NDT * 512)
        wp_t = W_proj[g * 512:(g + 1) * 512].reshape(NPAIR, 128, 1024).transpose(1, 0, 2).reshape(128, NPAIR * 1024)
        bq = b_qkv[g * 512:(g + 1) * 512].reshape(NPAIR, 128).T
        bk = b_qkv[1024 + g * 512:1024 + (g + 1) * 512].reshape(NPAIR, 128).T
        bqk = np.concatenate([bq, bk], axis=1)
        in_maps.append({
            "xt": np.ascontiguousarray(xt, np.float16),
            "wqk": np.ascontiguousarray(wqk, np.float16),
            "wv": np.ascontiguousarray(wv_t, np.float16),
            "wp": np.ascontiguousarray(wp_t, np.float16),
            "bqk": np.ascontiguousarray(bqk, np.float32),
        })
    return in_maps


def _assemble(results, b_qkv, W_proj, b_proj):
    const = (b_qkv[2048:3072].astype(np.float64) @ W_proj.astype(np.float64)).astype(np.float32) + b_proj
    parts = [results[c]["out"].reshape(S, D) for c in range(NCORES)]
    out = np.stack([parts[2 * b] + parts[2 * b + 1] + const for b in range(B)])
    return out.astype(np.float32)


def kernel(x, W_qkv, b_qkv, W_proj, b_proj, _reps=1):
    x = np.asarray(x, np.float32)
    W_qkv = np.asarray(W_qkv, np.float32)
    b_qkv = np.asarray(b_qkv, np.float32)
    W_proj = np.asarray(W_proj, np.float32)
    b_proj = np.asarray(b_proj, np.float32)
    assert x.shape == (B, S, D), x.shape
    run = _get_runner()
    in_maps = _prep_in_maps(x, W_qkv, b_qkv, W_proj)
    results, per_call = run(in_maps, reps=_reps)
    kernel.last_per_call = per_call
    return _assemble(results, b_qkv, W_proj, b_proj)


kernel.last_per_call = None
